# revision 7
# baseline (speedup 1.0000x reference)
"""Trainium2 Bass kernel: 12-head self-attention (B=8, N=1024, D=768).

Sharding: data-parallel over batch - one batch element per NeuronCore,
weights replicated on all 8 cores, no collectives.

Per-core dataflow (matmuls bf16 operands, fp32 PSUM accumulation).
Heads are processed in PAIRS (2p, 2p+1): head 2p lives on SBUF
partitions 0..63, head 2p+1 on 64..127 of the same qk chunk, so the
K=64 score matmuls of the two heads land in different PE row-groups
(row tiling) and execute CONCURRENTLY; the M=64 PV matmuls of the two
heads land in different PE column-groups (col tiling, out partitions
0..63 vs 64..127 of a shared accumulator) and also run concurrently.
Softmax denominators come from M=1 ones-matmuls batched 4 per window
into distinct 32-wide PSUM column strips (4-way concurrent), PSUM-
accumulated over all 8 key tiles, then combined via a DVE copy + tiny
DMA hops to partition 0 (DVE cannot cross partitions).

  xT [768,1024] (host-pretransposed, bf16)
  qkT[t] = W_qk[:,t-chunk].T @ xT             (feature-major q/k)
  v[mt]  = xT[:,mt-chunk].T @ W_v             (token-major v, pure)
  per pair p, per key tile mt:
    S^T_A = kT_A[:,mt].T @ qT_A   (rows 0:64,  row-tile 0, K=64)
    S^T_B = kT_B[:,mt].T @ qT_B   (rows 64:128, row-tile 64) - concurrent
    P_A = exp(scale*S^T_A); P_B likewise (ACT, no max-subtraction:
          scores are ~N(0,1), exp is safe in f32)
    psO[0:64]   += v_A[mt].T @ P_A  (col-tile 0)
    psO[64:128] += v_B[mt].T @ P_B  (col-tile 64) - concurrent
  pair end: 4 sums windows (ones.T @ P at col strips 0/32/64/96),
    then copy/ gather/ add/ reciprocal/ partition-broadcast/ multiply
    into attn_sb chunk p (rows already head-aligned, no DMA hop).
  proj split: chunks 0-2 run as PE filler once pairs 0-2 are normed
  (partials stashed bf16 with bias), chunks 3-5 + partial add at end.

Scheduling: software pipeline; 3 rotating PSUM slots (psA: ST_A, ST_B,
filler/sums) + 1 pair accumulator (psB); PV lags ST/exp by one mt; qkT
and v chains are interleaved as PE filler inside the pair loops.
"""

from contextlib import ExitStack

import numpy as np
import ml_dtypes

import concourse.bacc as bacc
import concourse.bass as bass
import concourse.mybir as mybir
import concourse.tile as tile
from concourse.bass_utils import run_bass_kernel_spmd

B, N, D = 8, 1024, 768
H, HD = 12, 64
NP = H // 2            # 6 head pairs
SCALE = HD ** -0.5
KC = D // 128          # 6 contraction chunks of 128
NT = N // 128          # 8 token tiles of 128
F32 = mybir.dt.float32
BF16 = mybir.dt.bfloat16
NCORES = 8
PC1 = 3                # proj phase-1 contraction chunks (0..PC1-1)

_CACHE = {}


def _build_nc():
    nc = bacc.Bacc(None, target_bir_lowering=False)
    xT = nc.dram_tensor("xT", [D, N], BF16, kind="ExternalInput")
    w_qk = nc.dram_tensor("w_qk", [D, 2 * D], BF16, kind="ExternalInput")
    w_v = nc.dram_tensor("w_v", [D, D], BF16, kind="ExternalInput")
    w_p = nc.dram_tensor("w_p", [D, D], BF16, kind="ExternalInput")
    bias = nc.dram_tensor("bias", [1, D], F32, kind="ExternalInput")
    out = nc.dram_tensor("out", [N, D], F32, kind="ExternalOutput")

    with ExitStack() as ctx:
        tc = ctx.enter_context(tile.TileContext(nc))
        const = ctx.enter_context(tc.tile_pool(name="const", bufs=1))
        work = ctx.enter_context(tc.tile_pool(name="work", bufs=2))
        # PSUM: 8 banks. psA = 3 rotating [128,1024] slots (ST_A/ST_B of the
        # current mt + one filler/sums slot) = 6 banks; psB = the pair's PV
        # accumulator = 2 banks.
        psA = ctx.enter_context(tc.tile_pool(name="psA", bufs=3, space="PSUM"))
        psB = ctx.enter_context(tc.tile_pool(name="psB", bufs=1, space="PSUM"))

        xT_sb = const.tile([128, KC, N], BF16)
        wqk_sb = const.tile([128, KC, 2 * D], BF16)
        wv_sb = const.tile([128, KC, D], BF16)
        wp_sb = const.tile([128, KC, D], BF16)
        bias_sb = const.tile([128, D], F32)
        qk_sb = const.tile([128, 2 * KC, N], BF16)   # chunks 0-5: qT, 6-11: kT
        v_sb = const.tile([128, NT, D], BF16)        # per-mt v, head-major
        attn_sb = const.tile([128, KC, N], BF16)     # attn_out^T, normalized
        opart_sb = const.tile([128, NT, D], BF16)    # proj partials (c 0..2)+bias
        ones_sb = const.tile([128, 1], BF16)

        # xT + W_qk per-chunk on the sync (HWDGE) queue so the first qkT
        # matmuls can start early; W_v/W_p/bias on the gpsimd queue.
        for c in range(KC):
            nc.sync.dma_start(out=xT_sb[:, c, :], in_=xT[128 * c:128 * (c + 1), :])
            nc.scalar.dma_start(out=wqk_sb[:, c, :], in_=w_qk[128 * c:128 * (c + 1), :])
        for c in range(KC):
            nc.gpsimd.dma_start(out=wv_sb[:, c, :], in_=w_v[128 * c:128 * (c + 1), :])
            nc.gpsimd.dma_start(out=wp_sb[:, c, :], in_=w_p[128 * c:128 * (c + 1), :])
        bap = bias[:, :]
        bias_bcast = bass.AP(
            tensor=bap.tensor, offset=bap.offset,
            ap=[[0, 128]] + list(bap.ap)[1:],
        )
        nc.gpsimd.dma_start(out=bias_sb, in_=bias_bcast)
        nc.gpsimd.memset(ones_sb, 1.0)

        v4 = v_sb.rearrange("p t (h e) -> p t h e", e=HD)

        def qkT_ops(t):
            """Closures: 6 accumulation-chunk matmul pairs + the cast copy,
            for interleaving as PE filler inside a pair's mt loop."""
            ps_qk = psA.tile([128, N], F32, tag="ps", name="ps_qk")
            ops = []
            for c in range(KC):
                def chunk(c=c, ps_qk=ps_qk):
                    for s in range(2):
                        nc.tensor.matmul(
                            ps_qk[:, 512 * s:512 * (s + 1)],
                            lhsT=wqk_sb[:, c, 128 * t:128 * (t + 1)],
                            rhs=xT_sb[:, c, 512 * s:512 * (s + 1)],
                            start=(c == 0), stop=(c == KC - 1),
                        )
                ops.append(chunk)

            def fin(ps_qk=ps_qk):
                nc.vector.tensor_copy(out=qk_sb[:, t, :], in_=ps_qk)
            ops.append(fin)
            return ops

        def v_ops(mt):
            ps_v = psA.tile([128, N], F32, tag="ps", name="ps_v")
            ops = []
            for c in range(KC):
                def chunk(c=c, ps_v=ps_v):
                    for lo, sz in ((0, 512), (512, 256)):
                        nc.tensor.matmul(
                            ps_v[:, lo:lo + sz],
                            lhsT=xT_sb[:, c, 128 * mt:128 * (mt + 1)],
                            rhs=wv_sb[:, c, lo:lo + sz],
                            start=(c == 0), stop=(c == KC - 1),
                        )
                ops.append(chunk)

            def fin(ps_v=ps_v):
                nc.vector.tensor_copy(
                    out=v4[:, mt, :, :],
                    in_=ps_v[:, 0:D].rearrange("p (h e) -> p h e", e=HD),
                )
            ops.append(fin)
            return ops

        def proj1_ops(nt):
            """Proj phase 1: contraction chunks 0..PC1-1 + bias, stashed bf16.
            Only legal once pairs 0..PC1-1 are normalized."""
            ps_p = psA.tile([128, N], F32, tag="ps", name="ps_p1")
            ops = []
            for c in range(PC1):
                def chunk(c=c, ps_p=ps_p):
                    for lo, sz in ((0, 512), (512, 256)):
                        nc.tensor.matmul(
                            ps_p[:, lo:lo + sz],
                            lhsT=attn_sb[:, c, 128 * nt:128 * (nt + 1)],
                            rhs=wp_sb[:, c, lo:lo + sz],
                            start=(c == 0), stop=(c == PC1 - 1),
                        )
                ops.append(chunk)

            def fin(ps_p=ps_p):
                nc.vector.tensor_add(
                    out=opart_sb[:, nt, :], in0=ps_p[:, 0:D], in1=bias_sb,
                )
            ops.append(fin)
            return ops

        def emit_proj2(nt):
            ps_p = psA.tile([128, N], F32, tag="ps", name="ps_p2")
            for c in range(PC1, KC):
                for lo, sz in ((0, 512), (512, 256)):
                    nc.tensor.matmul(
                        ps_p[:, lo:lo + sz],
                        lhsT=attn_sb[:, c, 128 * nt:128 * (nt + 1)],
                        rhs=wp_sb[:, c, lo:lo + sz],
                        start=(c == PC1), stop=(c == KC - 1),
                    )
            o_sb = work.tile([128, D], F32, tag="o_sb", name="o_sb")
            nc.vector.tensor_add(out=o_sb, in0=ps_p[:, 0:D], in1=opart_sb[:, nt, :])
            nc.sync.dma_start(out=out[128 * nt:128 * (nt + 1), :], in_=o_sb)

        def emit_ST_pair(p, mt):
            """Concurrent K=64 score matmuls for heads 2p (rows 0:64, PE row
            tile 0) and 2p+1 (rows 64:128, row tile 64), then the two exps."""
            tq, tk = p, KC + p
            ps_sA = psA.tile([128, N], F32, tag="ps", name="ps_sA")
            ps_sB = psA.tile([128, N], F32, tag="ps", name="ps_sB")
            for s in range(2):
                nc.tensor.matmul(
                    ps_sA[:, 512 * s:512 * (s + 1)],
                    lhsT=qk_sb[0:64, tk, 128 * mt:128 * (mt + 1)],
                    rhs=qk_sb[0:64, tq, 512 * s:512 * (s + 1)],
                    start=True, stop=True,
                )
                nc.tensor.matmul(
                    ps_sB[:, 512 * s:512 * (s + 1)],
                    lhsT=qk_sb[64:128, tk, 128 * mt:128 * (mt + 1)],
                    rhs=qk_sb[64:128, tq, 512 * s:512 * (s + 1)],
                    start=True, stop=True,
                )
            ptA = work.tile([128, N], BF16, tag="pt", name="ptA", bufs=16)
            ptB = work.tile([128, N], BF16, tag="pt", name="ptB", bufs=16)
            nc.scalar.activation(
                out=ptA, in_=ps_sA,
                func=mybir.ActivationFunctionType.Exp, scale=SCALE,
            )
            nc.scalar.activation(
                out=ptB, in_=ps_sB,
                func=mybir.ActivationFunctionType.Exp, scale=SCALE,
            )
            return ptA, ptB

        def emit_PV_pair(p, mt, ptA, ptB, ps_o):
            """Concurrent M=64 PV matmuls: head A -> out partitions 0:64 (PE
            col tile 0), head B -> 64:128 (col tile 64), shared accumulator."""
            hA, hB = 2 * p, 2 * p + 1
            for s in range(2):
                nc.tensor.matmul(
                    ps_o[0:64, 512 * s:512 * (s + 1)],
                    lhsT=v4[:, mt, hA, :],
                    rhs=ptA[:, 512 * s:512 * (s + 1)],
                    start=(mt == 0), stop=(mt == NT - 1),
                )
                nc.tensor.matmul(
                    ps_o[64:128, 512 * s:512 * (s + 1)],
                    lhsT=v4[:, mt, hB, :],
                    rhs=ptB[:, 512 * s:512 * (s + 1)],
                    start=(mt == 0), stop=(mt == NT - 1),
                )

        def emit_sums_window(ps_m, j, pts):
            """4-way concurrent column-strip sums: ones.T @ P for (A,2j)@0,
            (B,2j)@32, (A,2j+1)@64, (B,2j+1)@96, accumulated over windows."""
            ptA0, ptB0 = pts[2 * j]
            ptA1, ptB1 = pts[2 * j + 1]
            quads = ((0, ptA0), (32, ptB0), (64, ptA1), (96, ptB1))
            for s in range(2):
                for strip, pt in quads:
                    nc.tensor.matmul(
                        ps_m[strip:strip + 1, 512 * s:512 * (s + 1)],
                        lhsT=ones_sb[:, 0:1],
                        rhs=pt[:, 512 * s:512 * (s + 1)],
                        start=(j == 0), stop=(j == 3),
                        # auto-derive caps base_partition at 64; the 4th
                        # column strip needs an explicit position.
                        tile_position=(0, strip),
                    )

        def emit_norm(p, ps_o, ps_m):
            """Combine strip partials (A: rows 0+64, B: rows 32+96), invert,
            broadcast, normalize pair outputs into attn chunk p."""
            tq = p
            scp = work.tile([128, N], BF16, tag="scp", name="scp")
            nc.vector.tensor_copy(out=scp, in_=ps_m)
            gat = work.tile([1, 3, N], BF16, tag="gat", name="gat", bufs=1)
            nc.sync.dma_start(out=gat[0:1, 0, :], in_=scp[64:65, :])
            nc.sync.dma_start(out=gat[0:1, 1, :], in_=scp[32:33, :])
            nc.sync.dma_start(out=gat[0:1, 2, :], in_=scp[96:97, :])
            recinA = work.tile([1, N], F32, tag="recin", name="recinA", bufs=2)
            recinB = work.tile([1, N], F32, tag="recin", name="recinB", bufs=2)
            nc.vector.tensor_add(out=recinA, in0=scp[0:1, :], in1=gat[0:1, 0, :])
            nc.vector.tensor_add(out=recinB, in0=gat[0:1, 1, :], in1=gat[0:1, 2, :])
            recA = work.tile([1, N], F32, tag="rec", name="recA", bufs=2)
            recB = work.tile([1, N], F32, tag="rec", name="recB", bufs=2)
            nc.vector.reciprocal_approx_fast(out=recA, in_=recinA)
            nc.vector.reciprocal_approx_fast(out=recB, in_=recinB)
            rbA = work.tile([128, N], F32, tag="rb", name="rbA")
            rbB = work.tile([128, N], F32, tag="rb", name="rbB")
            nc.gpsimd.partition_broadcast(rbA, recA)
            nc.gpsimd.partition_broadcast(rbB, recB)
            nc.vector.tensor_mul(
                out=attn_sb[0:64, tq, :], in0=ps_o[0:64, :], in1=rbA[0:64, :],
            )
            nc.vector.tensor_mul(
                out=attn_sb[64:128, tq, :], in0=ps_o[64:128, :], in1=rbB[64:128, :],
            )

        # ---- schedule ----
        # Prologue: q/k chunks for pair 0 only; everything else is filler.
        for op in qkT_ops(0):
            op()
        for op in qkT_ops(KC):
            op()

        # PE filler per pair. Constraints: all v chains must complete within
        # pair 0 (pair-0 PVs consume them at mt pace); qkT chains for pair
        # p+1 must complete within pair p; proj phase 1 needs pairs 0..2.
        fillers = {p: [] for p in range(NP)}
        for mt in range(NT):
            fillers[0] += v_ops(mt)
        fillers[0] += qkT_ops(1) + qkT_ops(KC + 1)
        for p in range(1, NP - 1):
            fillers[p] = qkT_ops(p + 1) + qkT_ops(KC + p + 1)
        fillers[3] += proj1_ops(0) + proj1_ops(1)
        fillers[4] += proj1_ops(2) + proj1_ops(3) + proj1_ops(4)
        fillers[5] = proj1_ops(5) + proj1_ops(6) + proj1_ops(7)

        for p in range(NP):
            ps_o = psB.tile([128, N], F32, tag="pso", name="ps_o")
            fl = fillers[p]
            fi = 0
            pts = []
            pend = None
            for mt in range(NT):
                ptA, ptB = emit_ST_pair(p, mt)
                pts.append((ptA, ptB))
                if pend is not None:
                    emit_PV_pair(p, pend, pts[pend][0], pts[pend][1], ps_o)
                pend = mt
                want = min(len(fl), ((mt + 1) * len(fl) + NT - 2) // (NT - 1))
                while fi < want:
                    fl[fi]()
                    fi += 1
            # pair end: sums windows (w3 needs the last exps; sandwich the
            # last PV between w2 and w3 to absorb the exp latency).
            ps_m = psA.tile([128, N], F32, tag="ps", name="ps_m")
            emit_sums_window(ps_m, 0, pts)
            emit_sums_window(ps_m, 1, pts)
            emit_sums_window(ps_m, 2, pts)
            emit_PV_pair(p, pend, pts[pend][0], pts[pend][1], ps_o)
            emit_sums_window(ps_m, 3, pts)
            emit_norm(p, ps_o, ps_m)

        for nt in range(NT):
            emit_proj2(nt)

    nc.compile()
    return nc


def _get_nc():
    if "nc" not in _CACHE:
        _CACHE["nc"] = _build_nc()
    return _CACHE["nc"]


def _make_in_maps(x, W_qkv, W_proj, b_proj):
    bf = ml_dtypes.bfloat16
    x = np.asarray(x, dtype=np.float32)
    W_qkv = np.asarray(W_qkv, dtype=np.float32)
    W_proj = np.asarray(W_proj, dtype=np.float32)
    b_proj = np.asarray(b_proj, dtype=np.float32)
    w_qk = np.ascontiguousarray(W_qkv[:, :2 * D]).astype(bf)
    w_v = np.ascontiguousarray(W_qkv[:, 2 * D:]).astype(bf)
    w_p = W_proj.astype(bf)
    bias = b_proj.reshape(1, D)
    return [
        {
            "xT": np.ascontiguousarray(x[b].T).astype(bf),
            "w_qk": w_qk,
            "w_v": w_v,
            "w_p": w_p,
            "bias": bias,
        }
        for b in range(NCORES)
    ]


def run(x, W_qkv, W_proj, b_proj, trace=False):
    nc = _get_nc()
    in_maps = _make_in_maps(x, W_qkv, W_proj, b_proj)
    res = run_bass_kernel_spmd(nc, in_maps, core_ids=list(range(NCORES)), trace=trace)
    out = np.stack([res.results[b]["out"] for b in range(NCORES)], axis=0)
    return out.astype(np.float32), res


def kernel(x, W_qkv, W_proj, b_proj):
    out, _ = run(x, W_qkv, W_proj, b_proj, trace=False)
    return out


# revision 14
# speedup vs baseline: 1.0415x; 1.0415x over previous
"""Trainium2 Bass kernel: 12-head self-attention (B=8, N=1024, D=768).

Sharding: data-parallel over batch - one batch element per NeuronCore,
weights replicated on all 8 cores, no collectives.

Per-core dataflow (matmuls bf16 operands, fp32 PSUM accumulation).
Heads are processed in PAIRS (2p, 2p+1): head 2p lives on SBUF
partitions 0..63, head 2p+1 on 64..127 of the same qk chunk, so the
K=64 score matmuls of the two heads land in different PE row-groups
(row tiling) and execute CONCURRENTLY; the M=64 PV matmuls of the two
heads land in different PE column-groups (col tiling, out partitions
0..63 vs 64..127 of a shared accumulator) and also run concurrently.
Softmax denominators come from M=1 ones-matmuls batched 4 per window
into distinct 32-wide PSUM column strips (4-way concurrent), PSUM-
accumulated over all 8 key tiles, then combined via a DVE copy + tiny
DMA hops to partition 0 (DVE cannot cross partitions).

  xT [768,1024] (host-pretransposed, bf16)
  qkT[t] = W_qk[:,t-chunk].T @ xT             (feature-major q/k)
  v[mt]  = xT[:,mt-chunk].T @ W_v             (token-major v, pure)
  per pair p, per key tile mt:
    S^T_A = kT_A[:,mt].T @ qT_A   (rows 0:64,  row-tile 0, K=64)
    S^T_B = kT_B[:,mt].T @ qT_B   (rows 64:128, row-tile 64) - concurrent
    P_A = exp(scale*S^T_A); P_B likewise (ACT, no max-subtraction:
          scores are ~N(0,1), exp is safe in f32)
    psO[0:64]   += v_A[mt].T @ P_A  (col-tile 0)
    psO[64:128] += v_B[mt].T @ P_B  (col-tile 64) - concurrent
  pair end: 4 sums windows (ones.T @ P at col strips 0/32/64/96),
    then copy/ gather/ add/ reciprocal/ partition-broadcast/ multiply
    into attn_sb chunk p (rows already head-aligned, no DMA hop).
  proj split: chunks 0-2 run as PE filler once pairs 0-2 are normed
  (partials stashed bf16 with bias), chunks 3-5 + partial add at end.

Scheduling: software pipeline; 3 rotating PSUM slots (psA: ST_A, ST_B,
filler/sums) + 1 pair accumulator (psB); PV lags ST/exp by one mt; qkT
and v chains are interleaved as PE filler inside the pair loops.
"""

from contextlib import ExitStack

import numpy as np
import ml_dtypes

import concourse.bacc as bacc
import concourse.bass as bass
import concourse.mybir as mybir
import concourse.tile as tile
from concourse.bass_utils import run_bass_kernel_spmd

B, N, D = 8, 1024, 768
H, HD = 12, 64
NP = H // 2            # 6 head pairs
SCALE = HD ** -0.5
KC = D // 128          # 6 contraction chunks of 128
NT = N // 128          # 8 token tiles of 128
F32 = mybir.dt.float32
BF16 = mybir.dt.bfloat16
NCORES = 8
PC1 = 4                # proj phase-1 contraction chunks (0..PC1-1)

_CACHE = {}


def _build_nc():
    nc = bacc.Bacc(None, target_bir_lowering=False)
    xT = nc.dram_tensor("xT", [D, N], BF16, kind="ExternalInput")
    w_qk = nc.dram_tensor("w_qk", [D, 2 * D], BF16, kind="ExternalInput")
    w_v = nc.dram_tensor("w_v", [D, D], BF16, kind="ExternalInput")
    w_p = nc.dram_tensor("w_p", [D, D], BF16, kind="ExternalInput")
    bias = nc.dram_tensor("bias", [1, D], F32, kind="ExternalInput")
    out = nc.dram_tensor("out", [N, D], F32, kind="ExternalOutput")

    with ExitStack() as ctx:
        tc = ctx.enter_context(tile.TileContext(nc))
        const = ctx.enter_context(tc.tile_pool(name="const", bufs=1))
        work = ctx.enter_context(tc.tile_pool(name="work", bufs=2))
        # PSUM: 8 banks. psA = 3 rotating [128,1024] slots (ST_A/ST_B of the
        # current mt + one filler/sums slot) = 6 banks; psB = the pair's PV
        # accumulator = 2 banks.
        psA = ctx.enter_context(tc.tile_pool(name="psA", bufs=3, space="PSUM"))
        psB = ctx.enter_context(tc.tile_pool(name="psB", bufs=1, space="PSUM"))

        xT_sb = const.tile([128, KC, N], BF16)
        wqk_sb = const.tile([128, KC, 2 * D], BF16)
        wv_sb = const.tile([128, KC, D], BF16)
        wp_sb = const.tile([128, KC, D], BF16)
        bias_sb = const.tile([128, D], F32)
        qk_sb = const.tile([128, 2 * KC, N], BF16)   # chunks 0-5: qT, 6-11: kT
        v_sb = const.tile([128, NT, D], BF16)        # per-mt v, head-major
        attn_sb = const.tile([128, KC, N], BF16)     # attn_out^T, normalized
        opart_sb = const.tile([128, NT, D], BF16)    # proj partials (c 0..2)+bias
        ones_sb = const.tile([128, 1], BF16)

        # Input DMAs: xT on sync, wqk on scalar (both feed the prologue qkT
        # chains), wv FIRST on gpsimd (pair-0 v-chain fillers consume it from
        # ~mt 0), then wp/bias which are only needed by the proj phases.
        for c in range(KC):
            nc.sync.dma_start(out=xT_sb[:, c, :], in_=xT[128 * c:128 * (c + 1), :])
            nc.scalar.dma_start(out=wqk_sb[:, c, :], in_=w_qk[128 * c:128 * (c + 1), :])
            nc.gpsimd.dma_start(out=wv_sb[:, c, :], in_=w_v[128 * c:128 * (c + 1), :])
        for c in range(KC):
            nc.gpsimd.dma_start(out=wp_sb[:, c, :], in_=w_p[128 * c:128 * (c + 1), :])
        bap = bias[:, :]
        bias_bcast = bass.AP(
            tensor=bap.tensor, offset=bap.offset,
            ap=[[0, 128]] + list(bap.ap)[1:],
        )
        nc.gpsimd.dma_start(out=bias_sb, in_=bias_bcast)
        nc.gpsimd.memset(ones_sb, 1.0)
        # Selector weights for combining the sums strips on the PE: col 0
        # picks rows {0,64} (head A partials), col 1 rows {32,96} (head B).
        selw_sb = const.tile([128, 2], BF16)
        nc.gpsimd.memset(selw_sb, 0.0)
        nc.gpsimd.memset(selw_sb[0:1, 0:1], 1.0)
        nc.gpsimd.memset(selw_sb[64:65, 0:1], 1.0)
        nc.gpsimd.memset(selw_sb[32:33, 1:2], 1.0)
        nc.gpsimd.memset(selw_sb[96:97, 1:2], 1.0)

        v4 = v_sb.rearrange("p t (h e) -> p t h e", e=HD)

        def qkT_ops(t):
            """Closures: 6 accumulation-chunk matmul pairs + the cast copy,
            for interleaving as PE filler inside a pair's mt loop."""
            ps_qk = psA.tile([128, N], F32, tag="ps", name="ps_qk")
            ops = []
            for c in range(KC):
                def chunk(c=c, ps_qk=ps_qk):
                    for s in range(2):
                        nc.tensor.matmul(
                            ps_qk[:, 512 * s:512 * (s + 1)],
                            lhsT=wqk_sb[:, c, 128 * t:128 * (t + 1)],
                            rhs=xT_sb[:, c, 512 * s:512 * (s + 1)],
                            start=(c == 0), stop=(c == KC - 1),
                        )
                ops.append(chunk)

            def fin(ps_qk=ps_qk):
                nc.vector.tensor_copy(out=qk_sb[:, t, :], in_=ps_qk)
            ops.append(fin)
            return ops

        def v_ops(mt):
            ps_v = psA.tile([128, N], F32, tag="ps", name="ps_v")
            ops = []
            for c in range(KC):
                def chunk(c=c, ps_v=ps_v):
                    for lo, sz in ((0, 512), (512, 256)):
                        nc.tensor.matmul(
                            ps_v[:, lo:lo + sz],
                            lhsT=xT_sb[:, c, 128 * mt:128 * (mt + 1)],
                            rhs=wv_sb[:, c, lo:lo + sz],
                            start=(c == 0), stop=(c == KC - 1),
                        )
                ops.append(chunk)

            def fin(ps_v=ps_v):
                nc.vector.tensor_copy(
                    out=v4[:, mt, :, :],
                    in_=ps_v[:, 0:D].rearrange("p (h e) -> p h e", e=HD),
                )
            ops.append(fin)
            return ops

        def proj1_ops(nt):
            """Proj phase 1: contraction chunks 0..PC1-1 + bias, stashed bf16.
            Only legal once pairs 0..PC1-1 are normalized."""
            ps_p = psA.tile([128, N], F32, tag="ps", name="ps_p1")
            ops = []
            for c in range(PC1):
                def chunk(c=c, ps_p=ps_p):
                    for lo, sz in ((0, 512), (512, 256)):
                        nc.tensor.matmul(
                            ps_p[:, lo:lo + sz],
                            lhsT=attn_sb[:, c, 128 * nt:128 * (nt + 1)],
                            rhs=wp_sb[:, c, lo:lo + sz],
                            start=(c == 0), stop=(c == PC1 - 1),
                        )
                ops.append(chunk)

            def fin(ps_p=ps_p):
                nc.vector.tensor_add(
                    out=opart_sb[:, nt, :], in0=ps_p[:, 0:D], in1=bias_sb,
                )
            ops.append(fin)
            return ops

        def emit_proj2(nt):
            ps_p = psA.tile([128, N], F32, tag="ps", name="ps_p2")
            for c in range(PC1, KC):
                for lo, sz in ((0, 512), (512, 256)):
                    nc.tensor.matmul(
                        ps_p[:, lo:lo + sz],
                        lhsT=attn_sb[:, c, 128 * nt:128 * (nt + 1)],
                        rhs=wp_sb[:, c, lo:lo + sz],
                        start=(c == PC1), stop=(c == KC - 1),
                    )
            o_sb = work.tile([128, D], F32, tag="o_sb", name="o_sb")
            nc.vector.tensor_add(out=o_sb, in0=ps_p[:, 0:D], in1=opart_sb[:, nt, :])
            nc.sync.dma_start(out=out[128 * nt:128 * (nt + 1), :], in_=o_sb)

        def emit_ST_pair(p, mt):
            """Concurrent K=64 score matmuls for heads 2p (rows 0:64, PE row
            tile 0) and 2p+1 (rows 64:128, row tile 64), then the two exps."""
            tq, tk = p, KC + p
            ps_sA = psA.tile([128, N], F32, tag="ps", name="ps_sA")
            ps_sB = psA.tile([128, N], F32, tag="ps", name="ps_sB")
            for s in range(2):
                nc.tensor.matmul(
                    ps_sA[:, 512 * s:512 * (s + 1)],
                    lhsT=qk_sb[0:64, tk, 128 * mt:128 * (mt + 1)],
                    rhs=qk_sb[0:64, tq, 512 * s:512 * (s + 1)],
                    start=True, stop=True,
                )
                nc.tensor.matmul(
                    ps_sB[:, 512 * s:512 * (s + 1)],
                    lhsT=qk_sb[64:128, tk, 128 * mt:128 * (mt + 1)],
                    rhs=qk_sb[64:128, tq, 512 * s:512 * (s + 1)],
                    start=True, stop=True,
                )
            ptA = work.tile([128, N], BF16, tag="pt", name="ptA", bufs=16)
            ptB = work.tile([128, N], BF16, tag="pt", name="ptB", bufs=16)
            nc.scalar.activation(
                out=ptA, in_=ps_sA,
                func=mybir.ActivationFunctionType.Exp, scale=SCALE,
            )
            nc.scalar.activation(
                out=ptB, in_=ps_sB,
                func=mybir.ActivationFunctionType.Exp, scale=SCALE,
            )
            return ptA, ptB

        def emit_PV_pair(p, mt, ptA, ptB, ps_o):
            """Concurrent M=64 PV matmuls: head A -> out partitions 0:64 (PE
            col tile 0), head B -> 64:128 (col tile 64), shared accumulator."""
            hA, hB = 2 * p, 2 * p + 1
            for s in range(2):
                nc.tensor.matmul(
                    ps_o[0:64, 512 * s:512 * (s + 1)],
                    lhsT=v4[:, mt, hA, :],
                    rhs=ptA[:, 512 * s:512 * (s + 1)],
                    start=(mt == 0), stop=(mt == NT - 1),
                )
                nc.tensor.matmul(
                    ps_o[64:128, 512 * s:512 * (s + 1)],
                    lhsT=v4[:, mt, hB, :],
                    rhs=ptB[:, 512 * s:512 * (s + 1)],
                    start=(mt == 0), stop=(mt == NT - 1),
                )

        def emit_sums_window(ps_m, j, pts):
            """4-way concurrent column-strip sums: ones.T @ P for (A,2j)@0,
            (B,2j)@32, (A,2j+1)@64, (B,2j+1)@96, accumulated over windows."""
            ptA0, ptB0 = pts[2 * j]
            ptA1, ptB1 = pts[2 * j + 1]
            quads = ((0, ptA0), (32, ptB0), (64, ptA1), (96, ptB1))
            for s in range(2):
                for strip, pt in quads:
                    nc.tensor.matmul(
                        ps_m[strip:strip + 1, 512 * s:512 * (s + 1)],
                        lhsT=ones_sb[:, 0:1],
                        rhs=pt[:, 512 * s:512 * (s + 1)],
                        start=(j == 0), stop=(j == 3),
                        # auto-derive caps base_partition at 64; the 4th
                        # column strip needs an explicit position.
                        tile_position=(0, strip),
                    )

        def emit_norm(p, ps_o, ps_m, tail=()):
            """Combine strip partials on the PE: one M=2 selector matmul
            (col 0 = rows 0+64 -> head A, col 1 = rows 32+96 -> head B; the
            bf16 strip copy's garbage rows hit zero selector weights) writes
            back into ps_m rows 0:2 (no extra PSUM). One 2-row reciprocal at
            base partition 0, then broadcast A via gpsimd and B via a
            partition-stride-0 SBUF DMA, and normalize into attn chunk p.
            `tail` closure-lists are PE work independent of this pair,
            interleaved to cover the path latency."""
            tq = p
            ti = iter(tail)
            scp = work.tile([128, N], BF16, tag="scp", name="scp")
            nc.vector.tensor_copy(out=scp, in_=ps_m)
            for op in next(ti, []):
                op()
            for s in range(2):
                nc.tensor.matmul(
                    ps_m[0:2, 512 * s:512 * (s + 1)],
                    lhsT=selw_sb[:, 0:2],
                    rhs=scp[:, 512 * s:512 * (s + 1)],
                    start=True, stop=True,
                )
            for op in next(ti, []):
                op()
            rec2 = work.tile([2, N], F32, tag="rec", name="rec2", bufs=2)
            nc.vector.reciprocal_approx_fast(out=rec2, in_=ps_m[0:2, :])
            rbA = work.tile([128, N], F32, tag="rb", name="rbA")
            rbB = work.tile([128, N], F32, tag="rb", name="rbB")
            nc.gpsimd.partition_broadcast(rbA[0:64, :], rec2[0:1, :])
            # partition_broadcast reads from base partition 0 only; hop
            # head B's reciprocal row down from partition 1 via DMA first.
            recB = work.tile([1, N], F32, tag="recB", name="recB", bufs=2)
            nc.sync.dma_start(out=recB, in_=rec2[1:2, :])
            nc.gpsimd.partition_broadcast(rbB, recB)
            nc.vector.tensor_mul(
                out=attn_sb[0:64, tq, :], in0=ps_o[0:64, :], in1=rbA[0:64, :],
            )
            nc.vector.tensor_mul(
                out=attn_sb[64:128, tq, :], in0=ps_o[64:128, :], in1=rbB[64:128, :],
            )
            for t in ti:
                for op in t:
                    op()

        # ---- schedule ----
        # Prologue: q/k chunks for pair 0 only; everything else is filler.
        for op in qkT_ops(0):
            op()
        for op in qkT_ops(KC):
            op()

        # PE filler per pair. Constraints: all v chains must complete within
        # pair 0 (pair-0 PVs consume them at mt pace); qkT chains for pair
        # p+1 must complete within pair p; proj phase 1 (chunks 0..3) needs
        # norm(3), so it lands in pair 4 / pair 5 / the pair-5 norm tail.
        fillers = {p: [] for p in range(NP)}
        for mt in range(NT):
            fillers[0] += v_ops(mt)
        fillers[0] += qkT_ops(1) + qkT_ops(KC + 1)
        for p in range(1, NP - 1):
            fillers[p] = qkT_ops(p + 1) + qkT_ops(KC + p + 1)
        fillers[4] += proj1_ops(0) + proj1_ops(1) + proj1_ops(2)
        fillers[5] = proj1_ops(3)
        norm_tails = {p: () for p in range(NP)}
        norm_tails[5] = (
            proj1_ops(4),
            proj1_ops(5),
            proj1_ops(6) + proj1_ops(7),
        )

        LAG = 3  # PV trails ST/exp by 3 mt steps: hides pair-boundary
        #          ps_o reuse stalls and pads the sums windows at pair end.
        for p in range(NP):
            ps_o = psB.tile([128, N], F32, tag="pso", name="ps_o")
            fl = fillers[p]
            fi = 0
            pts = []
            for mt in range(NT):
                ptA, ptB = emit_ST_pair(p, mt)
                pts.append((ptA, ptB))
                if mt >= LAG:
                    k = mt - LAG
                    emit_PV_pair(p, k, pts[k][0], pts[k][1], ps_o)
                want = min(len(fl), (mt * len(fl) + NT - 2) // (NT - 1))
                while fi < want:
                    fl[fi]()
                    fi += 1
            while fi < len(fl):
                fl[fi]()
                fi += 1
            # pair end: sums windows sandwiched with the lagged PVs (w3
            # needs the last exps; the PVs absorb the exp latency).
            ps_m = psA.tile([128, N], F32, tag="ps", name="ps_m")
            emit_sums_window(ps_m, 0, pts)
            emit_PV_pair(p, NT - 3, pts[NT - 3][0], pts[NT - 3][1], ps_o)
            emit_sums_window(ps_m, 1, pts)
            emit_PV_pair(p, NT - 2, pts[NT - 2][0], pts[NT - 2][1], ps_o)
            emit_sums_window(ps_m, 2, pts)
            emit_PV_pair(p, NT - 1, pts[NT - 1][0], pts[NT - 1][1], ps_o)
            emit_sums_window(ps_m, 3, pts)
            emit_norm(p, ps_o, ps_m, tail=norm_tails[p])

        for nt in range(NT):
            emit_proj2(nt)

    nc.compile()
    return nc


def _get_nc():
    if "nc" not in _CACHE:
        _CACHE["nc"] = _build_nc()
    return _CACHE["nc"]


def _make_in_maps(x, W_qkv, W_proj, b_proj):
    bf = ml_dtypes.bfloat16
    x = np.asarray(x, dtype=np.float32)
    W_qkv = np.asarray(W_qkv, dtype=np.float32)
    W_proj = np.asarray(W_proj, dtype=np.float32)
    b_proj = np.asarray(b_proj, dtype=np.float32)
    w_qk = np.ascontiguousarray(W_qkv[:, :2 * D]).astype(bf)
    w_v = np.ascontiguousarray(W_qkv[:, 2 * D:]).astype(bf)
    w_p = W_proj.astype(bf)
    bias = b_proj.reshape(1, D)
    return [
        {
            "xT": np.ascontiguousarray(x[b].T).astype(bf),
            "w_qk": w_qk,
            "w_v": w_v,
            "w_p": w_p,
            "bias": bias,
        }
        for b in range(NCORES)
    ]


def run(x, W_qkv, W_proj, b_proj, trace=False):
    nc = _get_nc()
    in_maps = _make_in_maps(x, W_qkv, W_proj, b_proj)
    res = run_bass_kernel_spmd(nc, in_maps, core_ids=list(range(NCORES)), trace=trace)
    out = np.stack([res.results[b]["out"] for b in range(NCORES)], axis=0)
    return out.astype(np.float32), res


def kernel(x, W_qkv, W_proj, b_proj):
    out, _ = run(x, W_qkv, W_proj, b_proj, trace=False)
    return out
